# revision 15
# baseline (speedup 1.0000x reference)
"""BigramHash embedding lookup kernel for 8 Trainium2 NeuronCores.

Strategy (matches the row-sharded / all-to-all hint, with the all-to-all done
host-side since we receive full inputs):
  - Host computes bucket ids h = (prev_id * MULT + id) % NUM_BUCKETS.
  - The embedding table is sharded row-wise across the 8 cores
    (SHARD = 250001 rows each, last shard zero-padded).
  - Tokens are routed to the core that owns their bucket; each core gathers
    its tokens' rows via indirect DMA, projects to model dim on the tensor
    engine, and writes a [CAP, 1024] output slab.
  - Host scatters the per-core slabs back to the original token order.
"""

from contextlib import ExitStack

import ml_dtypes
import numpy as np

import concourse.bass as bass
import concourse.mybir as mybir
import concourse.tile as tile
from concourse import bacc
from concourse.bass import IndirectOffsetOnAxis
from concourse.bass_utils import run_bass_kernel_spmd
from concourse.masks import make_identity

NUM_BUCKETS = 2000003
HASH_DIM = 64
MODEL_DIM = 1024
HASH_MULT = 92821
N_CORES = 8
P = 128
SHARD = 250001  # ceil(NUM_BUCKETS / N_CORES); 8*250001 = 2000008 >= NUM_BUCKETS
GK = 8  # idx-grid columns (128 tokens each) per indirect-DMA gather chunk
NFREE = 512  # matmul moving-operand free dim (one PSUM bank of f32)

_prog_cache: dict[int, "bass.Bass"] = {}


def _build_program(K: int) -> "bass.Bass":
    """One SPMD program processing CAP = K*128 tokens from a SHARD-row table."""
    nc = bacc.Bacc(
        "TRN2", target_bir_lowering=False, debug=False, num_devices=N_CORES
    )
    f32 = mybir.dt.float32
    idx_d = nc.dram_tensor("idx", [P, K], mybir.dt.int32, kind="ExternalInput").ap()
    tab_d = nc.dram_tensor(
        "table", [SHARD, HASH_DIM], f32, kind="ExternalInput"
    ).ap()
    projT_d = nc.dram_tensor(
        "projT", [HASH_DIM, MODEL_DIM], mybir.dt.bfloat16, kind="ExternalInput"
    ).ap()
    out_d = nc.dram_tensor(
        "out", [P * K, MODEL_DIM], f32, kind="ExternalOutput"
    ).ap()

    with tile.TileContext(nc) as tc, ExitStack() as ctx:
        const_p = ctx.enter_context(tc.tile_pool(name="const", bufs=1))
        idx_p = ctx.enter_context(tc.tile_pool(name="idx", bufs=1))
        emb_p = ctx.enter_context(tc.tile_pool(name="emb", bufs=6))
        embT_p = ctx.enter_context(tc.tile_pool(name="embT", bufs=3))
        out_p = ctx.enter_context(tc.tile_pool(name="out", bufs=3))
        ps_t = ctx.enter_context(tc.tile_pool(name="ps_t", bufs=2, space="PSUM"))
        ps_mm = ctx.enter_context(tc.tile_pool(name="ps_mm", bufs=3, space="PSUM"))

        bf16 = mybir.dt.bfloat16
        ident = const_p.tile([P, P], f32)
        make_identity(nc, ident[:])
        # projT duplicated on partitions 0-63 and 64-127 so the paired
        # matmuls read lhsT/rhs from matching base partitions (row groups).
        projT_s = const_p.tile([P, MODEL_DIM], bf16)
        nc.sync.dma_start(out=projT_s[:HASH_DIM, :], in_=projT_d[:])
        nc.sync.dma_start(out=projT_s[HASH_DIM:, :], in_=projT_d[:])

        idx_t = idx_p.tile([P, K], mybir.dt.int32)
        nc.sync.dma_start(out=idx_t[:], in_=idx_d[:])
        for pb in range(0, K, 2):
            npair = min(2, K - pb)
            # HW indirect DMA: one offset per partition; each partition reads
            # dst-free-size contiguous elements -> exactly one 64-f32 row.
            embp = emb_p.tile([P, 2 * HASH_DIM], f32)
            for j in range(npair):
                nc.gpsimd.indirect_dma_start(
                    out=embp[:, j * HASH_DIM : (j + 1) * HASH_DIM],
                    out_offset=None,
                    in_=tab_d[:],
                    in_offset=IndirectOffsetOnAxis(
                        ap=idx_t[:, pb + j : pb + j + 1], axis=0
                    ),
                )
            # One PE transpose covers both blocks: rows [j*64,(j+1)*64) of the
            # result are block j's hash dims for its 128 tokens.
            eT_ps = ps_t.tile([npair * HASH_DIM, P], f32)
            nc.tensor.transpose(eT_ps[:], embp[:, : npair * HASH_DIM], ident[:])
            eT = embT_p.tile([npair * HASH_DIM, P], bf16)
            nc.vector.tensor_copy(eT[:], eT_ps[:])
            for j in range(npair):
                b = pb + j
                o_t = out_p.tile([P, MODEL_DIM], f32)
                mm = ps_mm.tile([P, MODEL_DIM], f32)
                for n in range(MODEL_DIM // NFREE):
                    nc.tensor.matmul(
                        mm[:, n * NFREE : (n + 1) * NFREE],
                        lhsT=eT[j * HASH_DIM : (j + 1) * HASH_DIM, :],
                        rhs=projT_s[
                            j * HASH_DIM : (j + 1) * HASH_DIM,
                            n * NFREE : (n + 1) * NFREE,
                        ],
                        start=True,
                        stop=True,
                    )
                if b % 2 == 0:
                    nc.vector.tensor_copy(o_t[:], mm[:])
                else:
                    nc.scalar.copy(o_t[:], mm[:])
                nc.sync.dma_start(out=out_d[b * P : (b + 1) * P, :], in_=o_t[:])
    nc.compile()
    return nc


def _route(input_ids: np.ndarray):
    """Bucket ids + routing of tokens to shard-owning cores."""
    ids = np.asarray(input_ids, dtype=np.int64)
    prev = np.empty_like(ids)
    prev[:, 0] = 0
    prev[:, 1:] = ids[:, :-1]
    h = ((prev * HASH_MULT + ids) % NUM_BUCKETS).reshape(-1)
    owner = h // SHARD
    local = (h - owner * SHARD).astype(np.int32)
    order = np.argsort(owner, kind="stable")
    counts = np.bincount(owner, minlength=N_CORES).astype(np.int64)
    return local, order, counts


def kernel(input_ids: np.ndarray, table: np.ndarray, proj_w: np.ndarray) -> np.ndarray:
    B, S = input_ids.shape
    T = B * S
    local, order, counts = _route(input_ids)
    sorted_local = local[order]
    offsets = np.zeros(N_CORES + 1, dtype=np.int64)
    np.cumsum(counts, out=offsets[1:])

    cap = max(P, int(-(-counts.max() // P)) * P)
    K = cap // P
    nc = _prog_cache.get(K)
    if nc is None:
        nc = _prog_cache.setdefault(K, _build_program(K))

    table = np.asarray(table, dtype=np.float32)
    projT = np.ascontiguousarray(
        np.asarray(proj_w, dtype=np.float32).T.astype(ml_dtypes.bfloat16)
    )
    in_maps = []
    for c in range(N_CORES):
        loc = sorted_local[offsets[c] : offsets[c + 1]]
        padded = np.zeros(cap, dtype=np.int32)
        padded[: counts[c]] = loc
        grid = np.ascontiguousarray(padded.reshape(K, P).T)
        lo = c * SHARD
        hi = min((c + 1) * SHARD, NUM_BUCKETS)
        shard = table[lo:hi]
        if hi - lo < SHARD:
            shard = np.concatenate(
                [shard, np.zeros((SHARD - (hi - lo), HASH_DIM), dtype=np.float32)]
            )
        in_maps.append(
            {
                "idx": grid,
                "table": np.ascontiguousarray(shard),
                "projT": projT,
            }
        )

    res = run_bass_kernel_spmd(nc, in_maps, list(range(N_CORES)))

    flat = np.empty((T, MODEL_DIM), dtype=np.float32)
    for c in range(N_CORES):
        flat[order[offsets[c] : offsets[c + 1]]] = res.results[c]["out"][: counts[c]]
    return flat.reshape(B, S, MODEL_DIM)
